# revision 1
# baseline (speedup 1.0000x reference)
"""GQA attention (B=2, S=2048, D=2048, H=32, G=8, hd=64) on 8 TRN2 cores.

Sharding: 2 batch groups x 4 TP ranks. Core c: batch b=c//4, rank r=c%4.
Each rank owns 2 KV groups (8 Q heads). All compute in bf16 (f32 PSUM accum).

Layout strategy (all transposes done on host):
  - x^T resident in SBUF; projections produce Q^T/K^T [feat, tok] and
    V [tok, feat] directly, so scores S^T [k, q] come out transpose-free and
    P^T blocks feed the PV matmul as the stationary operand with no on-chip
    transposes at all.
  - softmax denominator: V is augmented with a ones column, so the PV matmul
    accumulates sum_k(exp) in PSUM row 64 for free. 1/d via exp(-ln(d)).
  - per-rank o^T [512, 2048] + recip rows are AllGathered over the TP group;
    each rank then output-projects its own 512-token slice (division folded
    into the lhsT of the output projection).
"""

import sys

sys.path.insert(0, "/opt/trn_rl_repo")

import numpy as np
import ml_dtypes

import concourse.bass as bass
import concourse.tile as tile
from concourse import bacc, mybir
from concourse.bass_utils import run_bass_kernel_spmd

BF16 = ml_dtypes.bfloat16
B, S, D = 2, 2048, 2048
H, G, HD = 32, 8, 64
REP = H // G
N_CORES = 8
TP = 4
QF = 512   # q features per rank
KF = 128   # k/v features per rank
DC = D // 128  # 16 dim chunks
TOK = S // TP  # 512 output tokens per rank
RG = [[0, 1, 2, 3], [4, 5, 6, 7]]

_CACHE = {}


def _build():
    f32 = mybir.dt.float32
    bf16 = mybir.dt.bfloat16
    nc = bacc.Bacc("TRN2", target_bir_lowering=False, debug=False, num_devices=N_CORES)

    xt = nc.dram_tensor("xt", [128, DC, S], bf16, kind="ExternalInput").ap()
    wqt = nc.dram_tensor("wqt", [128, DC, QF], bf16, kind="ExternalInput").ap()
    wkt = nc.dram_tensor("wkt", [128, DC, KF], bf16, kind="ExternalInput").ap()
    wvt = nc.dram_tensor("wvt", [128, DC, 130], bf16, kind="ExternalInput").ap()
    cosr = nc.dram_tensor("cosr", [128, S], bf16, kind="ExternalInput").ap()
    sinr = nc.dram_tensor("sinr", [128, S], bf16, kind="ExternalInput").ap()
    wot = nc.dram_tensor("wot", [128, DC, 512], bf16, kind="ExternalInput").ap()
    out = nc.dram_tensor("out", [S, 512], f32, kind="ExternalOutput").ap()

    Exp = mybir.ActivationFunctionType.Exp
    Ln = mybir.ActivationFunctionType.Ln
    swap_mask = [i ^ 1 for i in range(32)]

    from contextlib import ExitStack
    with tile.TileContext(nc) as tc, ExitStack() as ctx:
        consts = ctx.enter_context(tc.tile_pool(name="consts", bufs=1))
        qk = ctx.enter_context(tc.tile_pool(name="qk", bufs=1))
        io = ctx.enter_context(tc.tile_pool(name="io", bufs=2))
        work = ctx.enter_context(tc.tile_pool(name="work", bufs=3))
        psum = ctx.enter_context(tc.tile_pool(name="psum", bufs=2, space="PSUM"))
        opsum = ctx.enter_context(tc.tile_pool(name="opsum", bufs=2, space="PSUM"))
        dram = ctx.enter_context(tc.tile_pool(name="dram", bufs=1, space="DRAM"))

        # ---- load inputs
        xt_sb = qk.tile([128, DC, S], bf16, tag="xt")
        nc.sync.dma_start(out=xt_sb[:], in_=xt[:])
        wqt_sb = consts.tile([128, DC, QF], bf16, tag="wqt")
        nc.sync.dma_start(out=wqt_sb[:], in_=wqt[:])
        wkt_sb = consts.tile([128, DC, KF], bf16, tag="wkt")
        nc.sync.dma_start(out=wkt_sb[:], in_=wkt[:])
        wvt_sb = consts.tile([128, DC, 130], bf16, tag="wvt")
        nc.sync.dma_start(out=wvt_sb[:], in_=wvt[:])
        cos_sb = consts.tile([128, S], bf16, tag="cos")
        nc.sync.dma_start(out=cos_sb[:], in_=cosr[:])
        sin_sb = consts.tile([128, S], bf16, tag="sin")
        nc.sync.dma_start(out=sin_sb[:], in_=sinr[:])

        # ---- projections: Q^T (4 j-tiles), K^T, V(+ones)
        qt_sb = qk.tile([128, 4, S], bf16, tag="qt")
        kt_sb = qk.tile([128, S], bf16, tag="kt")
        vaug_sb = qk.tile([128, DC, 130], bf16, tag="vaug")

        for j in range(4):
            for nt in range(4):
                ps = psum.tile([128, 512], mybir.dt.float32, tag="ps")
                for c in range(DC):
                    nc.tensor.matmul(
                        ps,
                        lhsT=wqt_sb[:, c, j * 128:(j + 1) * 128],
                        rhs=xt_sb[:, c, nt * 512:(nt + 1) * 512],
                        start=(c == 0),
                        stop=(c == DC - 1),
                    )
                nc.vector.tensor_copy(qt_sb[:, j, nt * 512:(nt + 1) * 512], ps)
        for nt in range(4):
            ps = psum.tile([128, 512], mybir.dt.float32, tag="ps")
            for c in range(DC):
                nc.tensor.matmul(
                    ps,
                    lhsT=wkt_sb[:, c, :],
                    rhs=xt_sb[:, c, nt * 512:(nt + 1) * 512],
                    start=(c == 0),
                    stop=(c == DC - 1),
                )
            nc.vector.tensor_copy(kt_sb[:, nt * 512:(nt + 1) * 512], ps)
        for tb in range(DC):
            ps = psum.tile([128, 512], mybir.dt.float32, tag="ps")
            for c in range(DC):
                nc.tensor.matmul(
                    ps[:, 0:130],
                    lhsT=xt_sb[:, c, tb * 128:(tb + 1) * 128],
                    rhs=wvt_sb[:, c, :],
                    start=(c == 0),
                    stop=(c == DC - 1),
                )
            nc.vector.tensor_copy(vaug_sb[:, tb, :], ps[:, 0:130])
        nc.vector.memset(vaug_sb[:, :, 64:65], 1.0)
        nc.vector.memset(vaug_sb[:, :, 129:130], 1.0)

        # ---- RoPE on Q^T and K^T (pair-swap shuffle + signed sin table)
        rope_tiles = [qt_sb[:, j, :] for j in range(4)] + [kt_sb[:, :]]
        for t in rope_tiles:
            sw = io.tile([128, S], bf16, tag="rsw")
            nc.vector.stream_shuffle(sw, t, swap_mask)
            nc.vector.tensor_mul(sw, sw, sin_sb[:])
            tmp = io.tile([128, S], bf16, tag="rtmp")
            nc.vector.tensor_mul(tmp, t, cos_sb[:])
            nc.vector.tensor_add(t, sw, tmp)

        # ---- attention
        # denom slots: head lh -> partition 32*(lh%4), free block lh//4
        denom_sb = consts.tile([97, 2 * S], mybir.dt.float32, tag="denom")
        nc.vector.memset(denom_sb[:], 1.0)
        scale = float(1.0 / np.sqrt(HD))

        # per-j chunked AllGathers so comm overlaps later j's compute.
        # AG1 chunk j: in [128, S] (rows 0:64 head 8r+j, 64:128 head 8r+4+j)
        #   -> out [512, S] rank-major. AG2 chunk j: recips [1, 2S] -> [4, 2S].
        ag1_ins, ag1_outs, ag2_ins, ag2_outs = [], [], [], []
        for j in range(4):
            t1 = dram.tile([128, S], bf16, tag=f"ag1i{j}", name=f"ag1i{j}")
            ag1_ins.append(t1)
            t2 = dram.tile([512, S], bf16, tag=f"ag1o{j}", name=f"ag1o{j}")
            ag1_outs.append(t2)
            t3 = dram.tile([1, 2 * S], mybir.dt.float32, tag=f"ag2i{j}", name=f"ag2i{j}")
            ag2_ins.append(t3)
            t4 = dram.tile([4, 2 * S], mybir.dt.float32, tag=f"ag2o{j}", name=f"ag2o{j}")
            ag2_outs.append(t4)
        wot_sb = qk.tile([128, DC, 512], bf16, tag="qt")  # reuse qt slot
        nc.sync.dma_start(out=wot_sb[:], in_=wot[:])

        for j in range(4):
            for qbp in range(2):
                q0 = qbp * 1024
                oA = opsum.tile([65, 1024], mybir.dt.float32, tag="ops")
                oB = opsum.tile([65, 1024], mybir.dt.float32, tag="ops")
                for kb in range(DC):
                    ksl = slice(kb * 128, (kb + 1) * 128)
                    sA = psum.tile([128, 1024], mybir.dt.float32, tag="ps")
                    sB = psum.tile([128, 1024], mybir.dt.float32, tag="ps")
                    for h in range(2):
                        qsl = slice(q0 + h * 512, q0 + (h + 1) * 512)
                        hs = slice(h * 512, (h + 1) * 512)
                        nc.tensor.matmul(
                            sA[:, hs], lhsT=kt_sb[0:64, ksl],
                            rhs=qt_sb[0:64, j, qsl],
                            start=True, stop=True, tile_position=(0, 0),
                        )
                        nc.tensor.matmul(
                            sB[:, hs], lhsT=kt_sb[64:128, ksl],
                            rhs=qt_sb[64:128, j, qsl],
                            start=True, stop=True, tile_position=(64, 0),
                        )
                    pA = work.tile([128, 1024], bf16, tag="pa")
                    pB = work.tile([128, 1024], bf16, tag="pb")
                    nc.scalar.activation(pA, sA, Exp, scale=scale)
                    nc.scalar.activation(pB, sB, Exp, scale=scale)
                    for h in range(2):
                        hs = slice(h * 512, (h + 1) * 512)
                        nc.tensor.matmul(
                            oA[:, hs], lhsT=vaug_sb[:, kb, 0:65], rhs=pA[:, hs],
                            start=(kb == 0), stop=(kb == DC - 1),
                        )
                        nc.tensor.matmul(
                            oB[:, hs], lhsT=vaug_sb[:, kb, 65:130], rhs=pB[:, hs],
                            start=(kb == 0), stop=(kb == DC - 1),
                        )
                otA = work.tile([64, 1024], bf16, tag="ot")
                nc.vector.tensor_copy(otA, oA[0:64, :])
                nc.sync.dma_start(
                    out=ag1_ins[j][0:64, q0:q0 + 1024], in_=otA)
                otB = work.tile([64, 1024], bf16, tag="ot")
                nc.vector.tensor_copy(otB, oB[0:64, :])
                nc.sync.dma_start(
                    out=ag1_ins[j][64:128, q0:q0 + 1024], in_=otB)
                nc.vector.tensor_copy(
                    denom_sb[32 * j:32 * j + 1, q0:q0 + 1024], oA[64:65, :])
                nc.vector.tensor_copy(
                    denom_sb[32 * j:32 * j + 1, S + q0:S + q0 + 1024],
                    oB[64:65, :])
            # reciprocal 1/d = exp(-ln(d)) for this j's two heads
            dsl = denom_sb[32 * j:32 * j + 1, :]
            nc.scalar.activation(dsl, dsl, Ln)
            nc.scalar.activation(dsl, dsl, Exp, scale=-1.0)
            nc.sync.dma_start(out=ag2_ins[j][:], in_=dsl)
            nc.gpsimd.collective_compute(
                "AllGather", mybir.AluOpType.bypass, replica_groups=RG,
                ins=[ag1_ins[j].opt()], outs=[ag1_outs[j].opt()],
            )
            nc.gpsimd.collective_compute(
                "AllGather", mybir.AluOpType.bypass, replica_groups=RG,
                ins=[ag2_ins[j].opt()], outs=[ag2_outs[j].opt()],
            )

        # ---- output projection, sharded by OUTPUT FEATURES (rank-dependence
        # lives in the per-core wot input shard: wo.T[:, r*512:(r+1)*512]).
        # Every core projects ALL 2048 tokens onto its 512 output features.
        # chunk c = 4j + r holds heads (8r+j | 8r+4+j); wot rows match.
        ot_sb = qk.tile([128, DC, S], bf16, tag="xt")  # reuse xt slot
        for c in range(DC):
            j, r = divmod(c, 4)
            nc.sync.dma_start(
                out=ot_sb[:, c, :], in_=ag1_outs[j][r * 128:(r + 1) * 128, :])
            r2 = io.tile([128, S], bf16, tag="r2")
            nc.gpsimd.dma_start(
                out=r2[0:64, :],
                in_=ag2_outs[j][r:r + 1, 0:S].partition_broadcast(64),
            )
            nc.gpsimd.dma_start(
                out=r2[64:128, :],
                in_=ag2_outs[j][r:r + 1, S:2 * S].partition_broadcast(64),
            )
            nc.vector.tensor_mul(ot_sb[:, c, :], ot_sb[:, c, :], r2)
        for tb in range(DC):
            ps = psum.tile([128, 512], mybir.dt.float32, tag="ps")
            for ic in range(DC):
                nc.tensor.matmul(
                    ps,
                    lhsT=ot_sb[:, ic, tb * 128:(tb + 1) * 128],
                    rhs=wot_sb[:, ic, :],
                    start=(ic == 0),
                    stop=(ic == DC - 1),
                )
            osb = work.tile([128, 512], mybir.dt.float32, tag="osb")
            nc.vector.tensor_copy(osb, ps)
            nc.sync.dma_start(out=out[tb * 128:(tb + 1) * 128, :], in_=osb)

    nc.compile()
    return nc


def _prep_inputs(x, freqs_cos, freqs_sin, wqkv, wo):
    """Build per-core input maps (host-side shard + transpose + bf16 cast)."""
    ins = []
    wo_t = np.ascontiguousarray(wo.T)  # [i, j]
    cos_h = np.empty((128, S), np.float32)
    sin_h = np.empty((128, S), np.float32)
    cs = freqs_cos[:, 0, :]  # [S, 64]
    sn = freqs_sin[:, 0, :]
    for p in range(128):
        cos_h[p] = cs[:, p % 64]
        sin_h[p] = sn[:, p % 64] * (-1.0 if p % 2 == 0 else 1.0)
    cos_h = cos_h.astype(BF16)
    sin_h = sin_h.astype(BF16)

    for c in range(N_CORES):
        b, r = divmod(c, TP)
        xt_h = np.ascontiguousarray(
            x[b].T.reshape(DC, 128, S).transpose(1, 0, 2)).astype(BF16)
        # Q rows, permuted: j-tile j = [head 8r+j | head 8r+4+j]
        rows = []
        for j in range(4):
            for h in (8 * r + j, 8 * r + 4 + j):
                rows.extend(range(h * HD, (h + 1) * HD))
        wq_sel = wqkv[rows, :]  # [512, D]
        wqt_h = np.ascontiguousarray(
            wq_sel.T.reshape(DC, 128, QF).transpose(1, 0, 2)).astype(BF16)
        krows = []
        for g in (2 * r, 2 * r + 1):
            krows.extend(range(H * HD + g * HD, H * HD + (g + 1) * HD))
        wk_sel = wqkv[krows, :]
        wkt_h = np.ascontiguousarray(
            wk_sel.T.reshape(DC, 128, KF).transpose(1, 0, 2)).astype(BF16)
        vrows = []
        for g in (2 * r, 2 * r + 1):
            vrows.extend(range((H + G) * HD + g * HD, (H + G) * HD + (g + 1) * HD))
        wv_sel = wqkv[vrows, :]  # [128, D]
        wvt_aug = np.zeros((D, 130), np.float32)
        wvt_aug[:, 0:64] = wv_sel[0:64].T
        wvt_aug[:, 65:129] = wv_sel[64:128].T
        wvt_h = np.ascontiguousarray(
            wvt_aug.reshape(DC, 128, 130).transpose(1, 0, 2)).astype(BF16)
        # wot rows permuted to match gathered chunk order c=4j+rr:
        # row c*128+p -> global i = 64*H + d, H = 8*rr + j + (4 if p>=64)
        perm = np.empty(D, np.int64)
        for cidx in range(DC):
            jj, rr = divmod(cidx, 4)
            for p in range(128):
                Hh = 8 * rr + jj + (4 if p >= 64 else 0)
                perm[cidx * 128 + p] = 64 * Hh + (p % 64)
        wot_h = np.ascontiguousarray(
            wo_t[perm][:, r * 512:(r + 1) * 512]
            .reshape(DC, 128, 512).transpose(1, 0, 2)).astype(BF16)
        ins.append({
            "xt": xt_h, "wqt": wqt_h, "wkt": wkt_h, "wvt": wvt_h,
            "cosr": cos_h, "sinr": sin_h, "wot": wot_h,
        })
    return ins


TRACE = False


def kernel(x, freqs_cos, freqs_sin, wqkv, wo):
    if "nc" not in _CACHE:
        _CACHE["nc"] = _build()
    nc = _CACHE["nc"]
    ins = _prep_inputs(x, freqs_cos, freqs_sin, wqkv, wo)
    res = run_bass_kernel_spmd(nc, ins, list(range(N_CORES)), trace=TRACE)
    _CACHE["res"] = res
    out = np.empty((B, S, D), np.float32)
    for c in range(N_CORES):
        b, r = divmod(c, TP)
        out[b, :, r * 512:(r + 1) * 512] = res.results[c]["out"]
    return out


if __name__ == "__main__":
    rng = np.random.default_rng(0)
    x = rng.normal(size=(B, S, D)).astype(np.float32)
    fc = rng.random(size=(S, 1, HD)).astype(np.float32)
    fs = rng.random(size=(S, 1, HD)).astype(np.float32)
    wq = rng.normal(size=(3072, D)).astype(np.float32) * 0.02
    wo = rng.normal(size=(D, D)).astype(np.float32) * 0.02
    o = kernel(x, fc, fs, wq, wo)
    print(o.shape, o.dtype)



# revision 5
# speedup vs baseline: 1.3926x; 1.3926x over previous
"""GQA attention (B=2, S=2048, D=2048, H=32, G=8, hd=64) on 8 TRN2 cores.

Sharding: 2 batch groups x 4 TP ranks, NO collectives. Core c: batch
b=c//4, rank r=c%4. Each rank owns 2 KV groups (8 Q heads), computes a
PARTIAL output projection over its 512 local head-features, and the host
sums the 4 rank partials per batch. This removes both AllGathers and the
PE stalls waiting on them.

Layout strategy (all transposes done on host):
  - x^T resident in SBUF; projections produce Q^T/K^T [feat, tok] and
    V [tok, feat] directly, so scores S^T [k, q] come out transpose-free
    and P^T blocks feed the PV matmul as the stationary operand.
  - softmax denominator: the PV stationary is [V_g (64) | ones (64)], so
    PSUM rows 64:128 accumulate sum_k(exp) replicated across 64
    partitions for free. Drain = vector reciprocal + fused multiply; no
    Ln/Exp table swaps, no partition-broadcast DMAs.
  - PSUM: scores [128,1024] double-buffered (4 banks) + o accumulators
    [128,1024] double-buffered (4 banks) = all 8 banks; the PE never
    waits on the Scalar engine's EXP.
  - q-token halves (qtile) are the outer loop so the output projection
    of half 0 overlaps attention of half 1.
"""

import sys

sys.path.insert(0, "/opt/trn_rl_repo")

import numpy as np
import ml_dtypes

import concourse.bass as bass
import concourse.tile as tile
from concourse import bacc, mybir
from concourse.bass_utils import run_bass_kernel_spmd

BF16 = ml_dtypes.bfloat16
B, S, D = 2, 2048, 2048
H, G, HD = 32, 8, 64
N_CORES = 8
TP = 4
QF = 512   # q features per rank
DC = D // 128  # 16 dim chunks
QT = 1024  # q tokens per attention pass (qtile)

_CACHE = {}


def _build():
    f32 = mybir.dt.float32
    bf16 = mybir.dt.bfloat16
    nc = bacc.Bacc("TRN2", target_bir_lowering=False, debug=False, num_devices=N_CORES)

    xt = nc.dram_tensor("xt", [128, DC, S], bf16, kind="ExternalInput").ap()
    wqt = nc.dram_tensor("wqt", [128, DC, QF], bf16, kind="ExternalInput").ap()
    wkt = nc.dram_tensor("wkt", [128, DC, 128], bf16, kind="ExternalInput").ap()
    wvt = nc.dram_tensor("wvt", [128, DC, 128], bf16, kind="ExternalInput").ap()
    cosr = nc.dram_tensor("cosr", [128, S], bf16, kind="ExternalInput").ap()
    sinr = nc.dram_tensor("sinr", [128, S], bf16, kind="ExternalInput").ap()
    wot = nc.dram_tensor("wot", [128, 4, D], bf16, kind="ExternalInput").ap()
    out = nc.dram_tensor("out", [S, D], f32, kind="ExternalOutput").ap()

    Exp = mybir.ActivationFunctionType.Exp
    swap_mask = [i ^ 1 for i in range(32)]
    scale = float(1.0 / np.sqrt(HD))

    from contextlib import ExitStack
    with tile.TileContext(nc) as tc, ExitStack() as ctx:
        consts = ctx.enter_context(tc.tile_pool(name="consts", bufs=1))
        qk = ctx.enter_context(tc.tile_pool(name="qk", bufs=1))
        io = ctx.enter_context(tc.tile_pool(name="io", bufs=2))
        work = ctx.enter_context(tc.tile_pool(name="work", bufs=3))
        dr = ctx.enter_context(tc.tile_pool(name="dr", bufs=2))
        ost = ctx.enter_context(tc.tile_pool(name="ost", bufs=4))
        psum = ctx.enter_context(tc.tile_pool(name="psum", bufs=2, space="PSUM"))
        opsum = ctx.enter_context(tc.tile_pool(name="opsum", bufs=2, space="PSUM"))

        # ---- load inputs (x^T split in 4 tiles so K-proj starts early)
        xt_sb = [
            qk.tile([128, 4, S], bf16, tag=f"xt{i}", name=f"xt_sb{i}")
            for i in range(4)
        ]
        wkt_sb = consts.tile([128, DC, 128], bf16, tag="wkt")
        nc.sync.dma_start(out=wkt_sb[:], in_=wkt[:])
        for i in range(4):
            nc.sync.dma_start(out=xt_sb[i][:], in_=xt[:, 4 * i:4 * i + 4, :])
        wvt_sb = consts.tile([128, DC, 128], bf16, tag="wvt")
        nc.sync.dma_start(out=wvt_sb[:], in_=wvt[:])
        cos_sb = consts.tile([128, S], bf16, tag="cos")
        nc.sync.dma_start(out=cos_sb[:], in_=cosr[:])
        sin_sb = consts.tile([128, S], bf16, tag="sin")
        nc.sync.dma_start(out=sin_sb[:], in_=sinr[:])
        wqt_sb = consts.tile([128, DC, QF], bf16, tag="wqt")
        nc.sync.dma_start(out=wqt_sb[:], in_=wqt[:])
        wot_sb = consts.tile([128, 4, D], bf16, tag="wot")
        nc.sync.dma_start(out=wot_sb[:], in_=wot[:])

        def xs(c):  # x^T chunk c as [128, 128or512...] slices
            return xt_sb[c // 4][:, c % 4, :]

        # ---- K^T projection [128 feat, S] then RoPE
        kt_sb = qk.tile([128, S], bf16, tag="kt")
        for nt in range(4):
            ps = psum.tile([128, 512], f32, tag="ps")
            for c in range(DC):
                nc.tensor.matmul(
                    ps, lhsT=wkt_sb[:, c, :],
                    rhs=xs(c)[:, nt * 512:(nt + 1) * 512],
                    start=(c == 0), stop=(c == DC - 1),
                )
            nc.vector.tensor_copy(kt_sb[:, nt * 512:(nt + 1) * 512], ps)
        sw = io.tile([128, S], bf16, tag="rsw")
        nc.vector.stream_shuffle(sw, kt_sb[:], swap_mask)
        nc.vector.tensor_mul(sw, sw, sin_sb[:])
        tmp = io.tile([128, S], bf16, tag="rtmp")
        nc.vector.tensor_mul(tmp, kt_sb[:], cos_sb[:])
        nc.vector.tensor_add(kt_sb[:], sw, tmp)

        # ---- V projection into PV-stationary layout:
        # vtile[:, tb, 0:64]=V_A, 64:128=ones, 128:192=V_B, 192:256=ones
        vtile = qk.tile([128, DC, 256], bf16, tag="vtile")
        nc.vector.memset(vtile[:, :, 64:128], 1.0)
        nc.vector.memset(vtile[:, :, 192:256], 1.0)
        for tb in range(DC):
            ps = psum.tile([128, 512], f32, tag="ps")
            for c in range(DC):
                nc.tensor.matmul(
                    ps[:, 0:128],
                    lhsT=xs(c)[:, tb * 128:(tb + 1) * 128],
                    rhs=wvt_sb[:, c, :],
                    start=(c == 0), stop=(c == DC - 1),
                )
            nc.vector.tensor_copy(vtile[:, tb, 0:64], ps[:, 0:64])
            nc.vector.tensor_copy(vtile[:, tb, 128:192], ps[:, 64:128])

        # ---- Q^T projection + RoPE, per (qtile, j)
        qt_sb = [
            qk.tile([128, 4, QT], bf16, tag=f"qt{q}", name=f"qt_sb{q}")
            for q in range(2)
        ]
        ot_sb = [
            qk.tile([128, 4, QT], bf16, tag=f"ot{q}", name=f"ot_sb{q}")
            for q in range(2)
        ]
        for q in range(2):
            for j in range(4):
                for nt in range(2):
                    ps = psum.tile([128, 512], f32, tag="ps")
                    tsl = slice(q * QT + nt * 512, q * QT + (nt + 1) * 512)
                    for c in range(DC):
                        nc.tensor.matmul(
                            ps, lhsT=wqt_sb[:, c, j * 128:(j + 1) * 128],
                            rhs=xs(c)[:, tsl],
                            start=(c == 0), stop=(c == DC - 1),
                        )
                    nc.vector.tensor_copy(
                        qt_sb[q][:, j, nt * 512:(nt + 1) * 512], ps)
                qsl = slice(q * QT, (q + 1) * QT)
                t = qt_sb[q][:, j, :]
                sw = io.tile([128, QT], bf16, tag="rsw")
                nc.vector.stream_shuffle(sw, t, swap_mask)
                nc.vector.tensor_mul(sw, sw, sin_sb[:, qsl])
                tmp = io.tile([128, QT], bf16, tag="rtmp")
                nc.vector.tensor_mul(tmp, t, cos_sb[:, qsl])
                nc.vector.tensor_add(t, sw, tmp)

        # ---- attention + per-qtile partial output projection
        for q in range(2):
            for j in range(4):
                for grp in range(2):
                    fsl = slice(64 * grp, 64 * (grp + 1))  # feat rows in kt/qt
                    vsl = slice(128 * grp, 128 * grp + 128)  # vtile cols
                    o = opsum.tile([128, QT], f32, tag="o")
                    for kb in range(DC):
                        ksl = slice(kb * 128, (kb + 1) * 128)
                        s = psum.tile([128, QT], f32, tag="ps")
                        for h in range(2):
                            hs = slice(h * 512, (h + 1) * 512)
                            nc.tensor.matmul(
                                s[:, hs], lhsT=kt_sb[fsl, ksl],
                                rhs=qt_sb[q][fsl, j, hs],
                                start=True, stop=True,
                            )
                        p = work.tile([128, QT], bf16, tag="p")
                        nc.scalar.activation(p, s, Exp, scale=scale)
                        for h in range(2):
                            hs = slice(h * 512, (h + 1) * 512)
                            nc.tensor.matmul(
                                o[:, hs], lhsT=vtile[:, kb, vsl], rhs=p[:, hs],
                                start=(kb == 0), stop=(kb == DC - 1),
                            )
                    # drain: rows 64:128 hold the denominator replicated;
                    # out head-rows = j-tile rows 64*grp:64*grp+64
                    rec = dr.tile([64, QT], f32, tag="rec")
                    nc.vector.reciprocal(rec, o[64:128, :])
                    nc.vector.tensor_mul(ot_sb[q][fsl, j, :], o[0:64, :], rec)
            # partial output projection for this qtile's tokens
            for tb in range(8):
                gtb = q * 8 + tb
                for od in range(4):
                    ps = psum.tile([128, 512], f32, tag="ps")
                    for ic in range(4):
                        nc.tensor.matmul(
                            ps,
                            lhsT=ot_sb[q][:, ic, tb * 128:(tb + 1) * 128],
                            rhs=wot_sb[:, ic, od * 512:(od + 1) * 512],
                            start=(ic == 0), stop=(ic == 3),
                        )
                    osb = ost.tile([128, 512], f32, tag="osb")
                    if od % 2 == 0:
                        nc.vector.tensor_copy(osb, ps)
                    else:
                        nc.scalar.activation(
                            osb, ps, mybir.ActivationFunctionType.Copy)
                    nc.sync.dma_start(
                        out=out[gtb * 128:(gtb + 1) * 128,
                                od * 512:(od + 1) * 512],
                        in_=osb)

    nc.compile()
    return nc


def _prep_inputs(x, freqs_cos, freqs_sin, wqkv, wo):
    """Build per-core input maps (host-side shard + transpose + bf16 cast)."""
    ins = []
    wo_t = np.ascontiguousarray(wo.T)  # [in feat, out feat]
    cos_h = np.empty((128, S), np.float32)
    sin_h = np.empty((128, S), np.float32)
    cs = freqs_cos[:, 0, :]  # [S, 64]
    sn = freqs_sin[:, 0, :]
    for p in range(128):
        cos_h[p] = cs[:, p % 64]
        sin_h[p] = sn[:, p % 64] * (-1.0 if p % 2 == 0 else 1.0)
    cos_h = cos_h.astype(BF16)
    sin_h = sin_h.astype(BF16)

    for core in range(N_CORES):
        b, r = divmod(core, TP)
        xt_h = np.ascontiguousarray(
            x[b].T.reshape(DC, 128, S).transpose(1, 0, 2)).astype(BF16)
        # Q rows, permuted: j-tile j = [head 8r+j | head 8r+4+j]
        rows = []
        for j in range(4):
            for h in (8 * r + j, 8 * r + 4 + j):
                rows.extend(range(h * HD, (h + 1) * HD))
        wq_sel = wqkv[rows, :]  # [512, D]
        wqt_h = np.ascontiguousarray(
            wq_sel.T.reshape(DC, 128, QF).transpose(1, 0, 2)).astype(BF16)
        krows = []
        for g in (2 * r, 2 * r + 1):
            krows.extend(range(H * HD + g * HD, H * HD + (g + 1) * HD))
        wk_sel = wqkv[krows, :]  # [128, D]
        wkt_h = np.ascontiguousarray(
            wk_sel.T.reshape(DC, 128, 128).transpose(1, 0, 2)).astype(BF16)
        vrows = []
        for g in (2 * r, 2 * r + 1):
            vrows.extend(range((H + G) * HD + g * HD, (H + G) * HD + (g + 1) * HD))
        wv_sel = wqkv[vrows, :]  # [128, D]; cols 0:64=V_A feats, 64:128=V_B
        wvt_h = np.ascontiguousarray(
            wv_sel.T.reshape(DC, 128, 128).transpose(1, 0, 2)).astype(BF16)
        # wot: local head-feature rows, chunk ic=j: [head 8r+j | head 8r+4+j]
        perm = np.empty(4 * 128, np.int64)
        for j in range(4):
            for p in range(128):
                Hh = 8 * r + j + (4 if p >= 64 else 0)
                perm[j * 128 + p] = 64 * Hh + (p % 64)
        wot_h = np.ascontiguousarray(
            wo_t[perm, :].reshape(4, 128, D).transpose(1, 0, 2)).astype(BF16)
        ins.append({
            "xt": xt_h, "wqt": wqt_h, "wkt": wkt_h, "wvt": wvt_h,
            "cosr": cos_h, "sinr": sin_h, "wot": wot_h,
        })
    return ins


TRACE = False


def kernel(x, freqs_cos, freqs_sin, wqkv, wo):
    if "nc" not in _CACHE:
        _CACHE["nc"] = _build()
    nc = _CACHE["nc"]
    ins = _prep_inputs(x, freqs_cos, freqs_sin, wqkv, wo)
    res = run_bass_kernel_spmd(nc, ins, list(range(N_CORES)), trace=TRACE)
    _CACHE["res"] = res
    out = np.empty((B, S, D), np.float32)
    for b in range(B):
        acc = res.results[TP * b]["out"].astype(np.float32)
        for r in range(1, TP):
            acc = acc + res.results[TP * b + r]["out"]
        out[b] = acc
    return out


if __name__ == "__main__":
    rng = np.random.default_rng(0)
    x = rng.normal(size=(B, S, D)).astype(np.float32)
    fc = rng.random(size=(S, 1, HD)).astype(np.float32)
    fs = rng.random(size=(S, 1, HD)).astype(np.float32)
    wq = rng.normal(size=(3072, D)).astype(np.float32) * 0.02
    wo = rng.normal(size=(D, D)).astype(np.float32) * 0.02
    o = kernel(x, fc, fs, wq, wo)
    print(o.shape, o.dtype)


# revision 8
# speedup vs baseline: 1.4714x; 1.0566x over previous
"""GQA attention (B=2, S=2048, D=2048, H=32, G=8, hd=64) on 8 TRN2 cores.

Sharding: 2 batch groups x 4 TP ranks, NO collectives. Core c: batch
b=c//4, rank r=c%4. Each rank owns 2 KV groups (8 Q heads), computes a
PARTIAL output projection over its 512 local head-features, and the host
sums the 4 rank partials per batch. This removes both AllGathers and the
PE stalls waiting on them.

Layout strategy (all transposes done on host):
  - x^T resident in SBUF; projections produce Q^T/K^T [feat, tok] and
    V [tok, feat] directly, so scores S^T [k, q] come out transpose-free
    and P^T blocks feed the PV matmul as the stationary operand.
  - softmax denominator: the PV stationary is [V_g (64) | ones (64)], so
    PSUM rows 64:128 accumulate sum_k(exp) replicated across 64
    partitions for free. Drain = vector reciprocal + fused multiply; no
    Ln/Exp table swaps, no partition-broadcast DMAs.
  - PSUM: scores [128,1024] double-buffered (4 banks) + o accumulators
    [128,1024] double-buffered (4 banks) = all 8 banks; the PE never
    waits on the Scalar engine's EXP.
  - q-token halves (qtile) are the outer loop so the output projection
    of half 0 overlaps attention of half 1.
"""

import sys

sys.path.insert(0, "/opt/trn_rl_repo")

import numpy as np
import ml_dtypes

import concourse.bass as bass
import concourse.tile as tile
from concourse import bacc, mybir
from concourse.bass_utils import run_bass_kernel_spmd

BF16 = ml_dtypes.bfloat16
B, S, D = 2, 2048, 2048
H, G, HD = 32, 8, 64
N_CORES = 8
TP = 4
QF = 512   # q features per rank
DC = D // 128  # 16 dim chunks
QT = 1024  # q tokens per attention pass (qtile)

_CACHE = {}


def _build():
    f32 = mybir.dt.float32
    bf16 = mybir.dt.bfloat16
    nc = bacc.Bacc("TRN2", target_bir_lowering=False, debug=False, num_devices=N_CORES)

    xt = nc.dram_tensor("xt", [128, DC, S], bf16, kind="ExternalInput").ap()
    wqt = nc.dram_tensor("wqt", [128, DC, QF], bf16, kind="ExternalInput").ap()
    wkt = nc.dram_tensor("wkt", [128, DC, 128], bf16, kind="ExternalInput").ap()
    wvt = nc.dram_tensor("wvt", [128, DC, 128], bf16, kind="ExternalInput").ap()
    cosr = nc.dram_tensor("cosr", [128, S], bf16, kind="ExternalInput").ap()
    sinr = nc.dram_tensor("sinr", [128, S], bf16, kind="ExternalInput").ap()
    wot = nc.dram_tensor("wot", [128, 4, D], bf16, kind="ExternalInput").ap()
    out = nc.dram_tensor("out", [S, D], f32, kind="ExternalOutput").ap()

    Exp = mybir.ActivationFunctionType.Exp
    swap_mask = [i ^ 1 for i in range(32)]
    scale = float(1.0 / np.sqrt(HD))

    from contextlib import ExitStack
    with tile.TileContext(nc) as tc, ExitStack() as ctx:
        consts = ctx.enter_context(tc.tile_pool(name="consts", bufs=1))
        qk = ctx.enter_context(tc.tile_pool(name="qk", bufs=1))
        io = ctx.enter_context(tc.tile_pool(name="io", bufs=2))
        work = ctx.enter_context(tc.tile_pool(name="work", bufs=3))
        dr = ctx.enter_context(tc.tile_pool(name="dr", bufs=2))
        ost = ctx.enter_context(tc.tile_pool(name="ost", bufs=4))
        psum = ctx.enter_context(tc.tile_pool(name="psum", bufs=2, space="PSUM"))
        opsum = ctx.enter_context(tc.tile_pool(name="opsum", bufs=4, space="PSUM"))

        # ---- load inputs (x^T split in 4 tiles so K-proj starts early)
        xt_sb = [
            qk.tile([128, 4, S], bf16, tag=f"xt{i}", name=f"xt_sb{i}")
            for i in range(4)
        ]
        wkt_sb = consts.tile([128, DC, 128], bf16, tag="wkt")
        nc.sync.dma_start(out=wkt_sb[:], in_=wkt[:])
        for i in range(4):
            nc.sync.dma_start(out=xt_sb[i][:], in_=xt[:, 4 * i:4 * i + 4, :])
        wvt_sb = consts.tile([128, DC, 128], bf16, tag="wvt")
        nc.sync.dma_start(out=wvt_sb[:], in_=wvt[:])
        cos_sb = consts.tile([128, S], bf16, tag="cos")
        nc.sync.dma_start(out=cos_sb[:], in_=cosr[:])
        sin_sb = consts.tile([128, S], bf16, tag="sin")
        nc.sync.dma_start(out=sin_sb[:], in_=sinr[:])
        wqt_sb = consts.tile([128, DC, QF], bf16, tag="wqt")
        nc.sync.dma_start(out=wqt_sb[:], in_=wqt[:])
        wot_sb = consts.tile([128, 4, D], bf16, tag="wot")
        nc.sync.dma_start(out=wot_sb[:], in_=wot[:])

        def xs(c):  # x^T chunk c as [128, 128or512...] slices
            return xt_sb[c // 4][:, c % 4, :]

        # ---- K^T projection [128 feat, S] then RoPE
        kt_sb = qk.tile([128, S], bf16, tag="kt")
        for nt in range(4):
            ps = psum.tile([128, 512], f32, tag="ps")
            for c in range(DC):
                nc.tensor.matmul(
                    ps, lhsT=wkt_sb[:, c, :],
                    rhs=xs(c)[:, nt * 512:(nt + 1) * 512],
                    start=(c == 0), stop=(c == DC - 1),
                )
            nc.vector.tensor_copy(kt_sb[:, nt * 512:(nt + 1) * 512], ps)
        sw = io.tile([128, S], bf16, tag="rsw")
        nc.vector.stream_shuffle(sw, kt_sb[:], swap_mask)
        nc.vector.tensor_mul(sw, sw, sin_sb[:])
        tmp = io.tile([128, S], bf16, tag="rtmp")
        nc.vector.tensor_mul(tmp, kt_sb[:], cos_sb[:])
        nc.vector.tensor_add(kt_sb[:], sw, tmp)

        # ---- V projection into PV-stationary layout:
        # vtile[:, tb, 0:64]=V_A, 64:128=ones, 128:192=V_B, 192:256=ones
        vtile = qk.tile([128, DC, 256], bf16, tag="vtile")
        nc.vector.memset(vtile[:, :, 64:128], 1.0)
        nc.vector.memset(vtile[:, :, 192:256], 1.0)
        for tb in range(DC):
            ps = psum.tile([128, 512], f32, tag="ps")
            for c in range(DC):
                nc.tensor.matmul(
                    ps[:, 0:128],
                    lhsT=xs(c)[:, tb * 128:(tb + 1) * 128],
                    rhs=wvt_sb[:, c, :],
                    start=(c == 0), stop=(c == DC - 1),
                )
            nc.vector.tensor_copy(vtile[:, tb, 0:64], ps[:, 0:64])
            nc.vector.tensor_copy(vtile[:, tb, 128:192], ps[:, 64:128])

        # ---- Q^T projection + RoPE, per (qtile, j)
        qt_sb = [
            qk.tile([128, 4, QT], bf16, tag=f"qt{q}", name=f"qt_sb{q}")
            for q in range(2)
        ]
        ot_sb = [
            qk.tile([128, 4, QT], bf16, tag=f"ot{q}", name=f"ot_sb{q}")
            for q in range(2)
        ]
        for q in range(2):
            for j in range(4):
                for nt in range(2):
                    ps = psum.tile([128, 512], f32, tag="ps")
                    tsl = slice(q * QT + nt * 512, q * QT + (nt + 1) * 512)
                    for c in range(DC):
                        nc.tensor.matmul(
                            ps, lhsT=wqt_sb[:, c, j * 128:(j + 1) * 128],
                            rhs=xs(c)[:, tsl],
                            start=(c == 0), stop=(c == DC - 1),
                        )
                    nc.vector.tensor_copy(
                        qt_sb[q][:, j, nt * 512:(nt + 1) * 512], ps)
                qsl = slice(q * QT, (q + 1) * QT)
                t = qt_sb[q][:, j, :]
                sw = io.tile([128, QT], bf16, tag="rsw")
                nc.vector.stream_shuffle(sw, t, swap_mask)
                nc.vector.tensor_mul(sw, sw, sin_sb[:, qsl])
                tmp = io.tile([128, QT], bf16, tag="rtmp")
                nc.vector.tensor_mul(tmp, t, cos_sb[:, qsl])
                nc.vector.tensor_add(t, sw, tmp)

        # ---- attention + per-qtile partial output projection
        # score pair: grp A on PE rows 0:64, grp B on rows 64:128 — packed
        # via tile_position so both matmuls run concurrently; their outputs
        # land in different PSUM banks (cols 0:512 / 512:1024 of sp).
        for q in range(2):
            for j in range(4):
                for qh in range(2):
                    qsl = slice(qh * 512, (qh + 1) * 512)
                    oA = opsum.tile([128, 512], f32, tag="o", name="oA")
                    oB = opsum.tile([128, 512], f32, tag="o", name="oB")
                    for kb in range(DC):
                        ksl = slice(kb * 128, (kb + 1) * 128)
                        sp = psum.tile([128, QT], f32, tag="ps")
                        nc.tensor.matmul(
                            sp[:, 0:512], lhsT=kt_sb[0:64, ksl],
                            rhs=qt_sb[q][0:64, j, qsl],
                            start=True, stop=True, tile_position=(0, 0),
                        )
                        nc.tensor.matmul(
                            sp[:, 512:1024], lhsT=kt_sb[64:128, ksl],
                            rhs=qt_sb[q][64:128, j, qsl],
                            start=True, stop=True, tile_position=(64, 0),
                        )
                        p = work.tile([128, QT], bf16, tag="p")
                        nc.scalar.activation(p, sp, Exp, scale=scale)
                        nc.tensor.matmul(
                            oA, lhsT=vtile[:, kb, 0:128], rhs=p[:, 0:512],
                            start=(kb == 0), stop=(kb == DC - 1),
                        )
                        nc.tensor.matmul(
                            oB, lhsT=vtile[:, kb, 128:256], rhs=p[:, 512:1024],
                            start=(kb == 0), stop=(kb == DC - 1),
                        )
                    # drain: rows 64:128 hold the denominator replicated
                    recA = dr.tile([64, 512], f32, tag="rec", name="recA")
                    nc.vector.reciprocal(recA, oA[64:128, :])
                    nc.vector.tensor_mul(
                        ot_sb[q][0:64, j, qsl], oA[0:64, :], recA)
                    recB = dr.tile([64, 512], f32, tag="rec", name="recB")
                    nc.vector.reciprocal(recB, oB[64:128, :])
                    nc.vector.tensor_mul(
                        ot_sb[q][64:128, j, qsl], oB[0:64, :], recB)
            # partial output projection for this qtile's tokens
            for tb in range(8):
                gtb = q * 8 + tb
                for od in range(4):
                    ps = psum.tile([128, 512], f32, tag="ps")
                    for ic in range(4):
                        nc.tensor.matmul(
                            ps,
                            lhsT=ot_sb[q][:, ic, tb * 128:(tb + 1) * 128],
                            rhs=wot_sb[:, ic, od * 512:(od + 1) * 512],
                            start=(ic == 0), stop=(ic == 3),
                        )
                    osb = ost.tile([128, 512], f32, tag="osb")
                    if od % 2 == 0:
                        nc.vector.tensor_copy(osb, ps)
                    else:
                        nc.scalar.activation(
                            osb, ps, mybir.ActivationFunctionType.Copy)
                    nc.sync.dma_start(
                        out=out[gtb * 128:(gtb + 1) * 128,
                                od * 512:(od + 1) * 512],
                        in_=osb)

    nc.compile()
    return nc


def _prep_inputs(x, freqs_cos, freqs_sin, wqkv, wo):
    """Build per-core input maps (host-side shard + transpose + bf16 cast)."""
    ins = []
    wo_t = np.ascontiguousarray(wo.T)  # [in feat, out feat]
    cos_h = np.empty((128, S), np.float32)
    sin_h = np.empty((128, S), np.float32)
    cs = freqs_cos[:, 0, :]  # [S, 64]
    sn = freqs_sin[:, 0, :]
    for p in range(128):
        cos_h[p] = cs[:, p % 64]
        sin_h[p] = sn[:, p % 64] * (-1.0 if p % 2 == 0 else 1.0)
    cos_h = cos_h.astype(BF16)
    sin_h = sin_h.astype(BF16)

    for core in range(N_CORES):
        b, r = divmod(core, TP)
        xt_h = np.ascontiguousarray(
            x[b].T.reshape(DC, 128, S).transpose(1, 0, 2)).astype(BF16)
        # Q rows, permuted: j-tile j = [head 8r+j | head 8r+4+j]
        rows = []
        for j in range(4):
            for h in (8 * r + j, 8 * r + 4 + j):
                rows.extend(range(h * HD, (h + 1) * HD))
        wq_sel = wqkv[rows, :]  # [512, D]
        wqt_h = np.ascontiguousarray(
            wq_sel.T.reshape(DC, 128, QF).transpose(1, 0, 2)).astype(BF16)
        krows = []
        for g in (2 * r, 2 * r + 1):
            krows.extend(range(H * HD + g * HD, H * HD + (g + 1) * HD))
        wk_sel = wqkv[krows, :]  # [128, D]
        wkt_h = np.ascontiguousarray(
            wk_sel.T.reshape(DC, 128, 128).transpose(1, 0, 2)).astype(BF16)
        vrows = []
        for g in (2 * r, 2 * r + 1):
            vrows.extend(range((H + G) * HD + g * HD, (H + G) * HD + (g + 1) * HD))
        wv_sel = wqkv[vrows, :]  # [128, D]; cols 0:64=V_A feats, 64:128=V_B
        wvt_h = np.ascontiguousarray(
            wv_sel.T.reshape(DC, 128, 128).transpose(1, 0, 2)).astype(BF16)
        # wot: local head-feature rows, chunk ic=j: [head 8r+j | head 8r+4+j]
        perm = np.empty(4 * 128, np.int64)
        for j in range(4):
            for p in range(128):
                Hh = 8 * r + j + (4 if p >= 64 else 0)
                perm[j * 128 + p] = 64 * Hh + (p % 64)
        wot_h = np.ascontiguousarray(
            wo_t[perm, :].reshape(4, 128, D).transpose(1, 0, 2)).astype(BF16)
        ins.append({
            "xt": xt_h, "wqt": wqt_h, "wkt": wkt_h, "wvt": wvt_h,
            "cosr": cos_h, "sinr": sin_h, "wot": wot_h,
        })
    return ins


TRACE = False


def kernel(x, freqs_cos, freqs_sin, wqkv, wo):
    if "nc" not in _CACHE:
        _CACHE["nc"] = _build()
    nc = _CACHE["nc"]
    ins = _prep_inputs(x, freqs_cos, freqs_sin, wqkv, wo)
    res = run_bass_kernel_spmd(nc, ins, list(range(N_CORES)), trace=TRACE)
    _CACHE["res"] = res
    out = np.empty((B, S, D), np.float32)
    for b in range(B):
        acc = res.results[TP * b]["out"].astype(np.float32)
        for r in range(1, TP):
            acc = acc + res.results[TP * b + r]["out"]
        out[b] = acc
    return out


if __name__ == "__main__":
    rng = np.random.default_rng(0)
    x = rng.normal(size=(B, S, D)).astype(np.float32)
    fc = rng.random(size=(S, 1, HD)).astype(np.float32)
    fs = rng.random(size=(S, 1, HD)).astype(np.float32)
    wq = rng.normal(size=(3072, D)).astype(np.float32) * 0.02
    wo = rng.normal(size=(D, D)).astype(np.float32) * 0.02
    o = kernel(x, fc, fs, wq, wo)
    print(o.shape, o.dtype)
